# revision 7
# baseline (speedup 1.0000x reference)
"""Trainium2 Bass kernel for nn_ContrastiveLoss (CE + pos-pair cosine + first-k
neg-pair cosine), SPMD across 8 NeuronCores.

Math used (mathematically identical to the reference, avoids the BxB cosine
matrix entirely):
  loss1 = mean_i( log(sum_c exp(yb[i,c])) - yb[i, y_i] )
  loss2: sum over same-label pairs i<j of cos_ij
           = (sum_c ||S_c||^2 - sum_i ||u_i||^2) / 2,   S_c = sum_{y_i=c} u_i,
         with u_i = x_i/||x_i||, sum_i ||u_i||^2 = B.
  loss3 needs only cos(x_0, x_j): the first n_neg (<= C = 1000) negative pairs
        in lexicographic (i<j) order all come from row i=0 whenever row 0 has
        >= n_neg differing-label partners (host-verified, else exact host
        fallback).

Sharding: labels are snake-packed onto 8 cores (125 labels/core); xs rows are
permuted so a core owns all rows of its labels -> per-core sum_c ||S_c||^2
partials are scalars (no cross-core reduction / no collectives). y_bars is
row-sharded naturally. Host glue is only O(B) index math plus summing 8 small
output vectors.
"""

import os
import numpy as np

try:
    import concourse.bass as bass  # noqa: F401
except ImportError:
    import sys

    for p in ("/opt/trn_rl_repo", "/root/.axon_site/_ro/trn_rl_repo"):
        if os.path.isdir(p):
            sys.path.insert(0, p)
            break
    import concourse.bass as bass  # noqa: F401

import concourse.tile as tile
from concourse import bacc, mybir
from concourse.bass_utils import run_bass_kernel_spmd

B, D, C, NCORES = 4096, 512, 1000, 8
RCE = B // NCORES  # 512 CE rows per core
PAD = 640          # padded loss2 row capacity per core (5 x 128)
LBL = 128          # label slots per core (1000/8 = 125 used)

F32 = mybir.dt.float32
BF16 = mybir.dt.bfloat16
ALU = mybir.AluOpType
ACTF = mybir.ActivationFunctionType

LAST_EXEC_NS = None  # filled when BASS_TRACE=1 and profiling succeeds

_PROGRAM = None  # cached (nc) build


def _build_program():
    """One SPMD program, identical on all 8 cores; all per-core variation is
    carried by the input tensors."""
    nc = bacc.Bacc("TRN2", target_bir_lowering=False, debug=False,
                   num_devices=NCORES)

    yb = nc.declare_dram_parameter("yb", [RCE, C], F32, isOutput=False)
    ycls = nc.declare_dram_parameter("ycls", [RCE, 1], F32, isOutput=False)
    xsp = nc.declare_dram_parameter("xsp", [PAD, D], F32, isOutput=False)
    yrel = nc.declare_dram_parameter("yrel", [PAD, 1], F32, isOutput=False)
    u0b = nc.declare_dram_parameter("u0b", [128, D], F32, isOutput=False)

    ce = nc.declare_dram_parameter("ce", [RCE, 1], F32, isOutput=True)
    snorm = nc.declare_dram_parameter("snorm", [128, 1], F32, isOutput=True)
    cos0 = nc.declare_dram_parameter("cos0", [PAD, 1], F32, isOutput=True)

    with tile.TileContext(nc) as tc:
        with (
            tc.tile_pool(name="const", bufs=1) as const,
            tc.tile_pool(name="io", bufs=3) as io,
            tc.tile_pool(name="trash", bufs=2) as trash,
            tc.tile_pool(name="small", bufs=8) as small,
            tc.tile_pool(name="psum", bufs=1, space="PSUM") as psum,
        ):
            iota_c = const.tile([128, C], F32)
            nc.gpsimd.iota(iota_c[:], [[1, C]], channel_multiplier=0,
                           allow_small_or_imprecise_dtypes=True)
            iota_l = const.tile([128, LBL], F32)
            nc.gpsimd.iota(iota_l[:], [[1, LBL]], channel_multiplier=0,
                           allow_small_or_imprecise_dtypes=True)
            u0t = const.tile([128, D], F32)
            nc.sync.dma_start(u0t[:], u0b[:])

            # ---- loss1: cross entropy over local y_bars rows -------------
            for i in range(RCE // 128):
                r = bass.ts(i, 128)
                t = io.tile([128, C], F32)
                nc.sync.dma_start(t[:], yb[r, :])
                yc = small.tile([128, 1], F32)
                nc.sync.dma_start(yc[:], ycls[r, :])

                ex = trash.tile([128, C], BF16)
                se = small.tile([128, 1], F32)
                nc.scalar.activation(ex[:], t[:], ACTF.Exp, accum_out=se[:])

                pk = small.tile([128, 1], F32)
                tt = trash.tile([128, C], F32)
                nc.vector.scalar_tensor_tensor(
                    tt[:], iota_c[:], yc[:], t[:],
                    op0=ALU.is_equal, op1=ALU.mult, accum_out=pk[:])

                lse = small.tile([128, 1], F32)
                nc.scalar.activation(lse[:], se[:], ACTF.Ln)
                cer = small.tile([128, 1], F32)
                nc.vector.tensor_sub(cer[:], lse[:], pk[:])
                nc.sync.dma_start(ce[r, :], cer[:])

            # ---- loss2 (label sums) + loss3 (row-0 cosine) ---------------
            S = psum.tile([128, D], F32)
            nsteps = PAD // 128
            for i in range(nsteps):
                r = bass.ts(i, 128)
                xt = io.tile([128, D], F32)
                nc.sync.dma_start(xt[:], xsp[r, :])
                yr = small.tile([128, 1], F32)
                nc.sync.dma_start(yr[:], yrel[r, :])

                nq = small.tile([128, 1], F32)
                td = trash.tile([128, D], F32)
                nc.vector.scalar_tensor_tensor(
                    td[:], xt[:], 1.0, xt[:],
                    op0=ALU.mult, op1=ALU.mult, accum_out=nq[:])
                # floor |x|^2 at 1e-12: zero-padded rows get inv_n = 1e6 but
                # contribute x*inv_n = 0 everywhere downstream.
                nc.vector.tensor_scalar_max(nq[:], nq[:], 1e-12)
                nrm = small.tile([128, 1], F32)
                nc.scalar.activation(nrm[:], nq[:], ACTF.Sqrt)
                invn = small.tile([128, 1], F32)
                nc.vector.reciprocal(invn[:], nrm[:])

                # scaled one-hot: oh[p, l] = (l == yrel_p) * inv_n_p, so the
                # matmul S = oh^T @ x accumulates already-normalized rows.
                oh = small.tile([128, LBL], F32)
                nc.vector.tensor_scalar(
                    oh[:], iota_l[:], yr[:], invn[:],
                    op0=ALU.is_equal, op1=ALU.mult)
                nc.tensor.matmul(S[:], oh[:], xt[:],
                                 start=(i == 0), stop=(i == nsteps - 1))

                # cos(x_0, x_j) = (x_j . u0) * inv_n_j for local rows
                dt0 = small.tile([128, 1], F32)
                td2 = trash.tile([128, D], F32)
                nc.vector.scalar_tensor_tensor(
                    td2[:], xt[:], 1.0, u0t[:],
                    op0=ALU.mult, op1=ALU.mult, accum_out=dt0[:])
                c0 = small.tile([128, 1], F32)
                nc.vector.tensor_mul(c0[:], dt0[:], invn[:])
                nc.sync.dma_start(cos0[r, :], c0[:])

            # DVE may read only one PSUM operand per instruction
            Ssb = io.tile([128, D], F32)
            nc.vector.tensor_copy(Ssb[:], S[:])
            sn = small.tile([128, 1], F32)
            td3 = trash.tile([128, D], F32)
            nc.vector.scalar_tensor_tensor(
                td3[:], Ssb[:], 1.0, Ssb[:],
                op0=ALU.mult, op1=ALU.mult, accum_out=sn[:])
            nc.sync.dma_start(snorm[:], sn[:])

    nc.compile()
    return nc


def _get_program():
    global _PROGRAM
    if _PROGRAM is None:
        _PROGRAM = _build_program()
    return _PROGRAM


def _shard(xs, y_bars, y):
    """Snake-pack labels onto cores; permute xs rows into per-core blocks.

    Returns in_maps plus the bookkeeping needed to reassemble outputs, or
    None if capacity assumptions fail (handled by exact host fallback).
    """
    counts = np.bincount(y, minlength=C).astype(np.int64)
    order = np.argsort(-counts, kind="stable")  # labels by count desc
    core_of_label = np.empty(C, np.int32)
    local_idx = np.empty(C, np.int32)
    nlab = np.zeros(NCORES, np.int32)
    for r in range(0, C, NCORES):
        blk = order[r:r + NCORES]
        cores = range(NCORES) if (r // NCORES) % 2 == 0 else range(NCORES - 1, -1, -1)
        for lab, core in zip(blk, cores):
            core_of_label[lab] = core
            local_idx[lab] = nlab[core]
            nlab[core] += 1
    if nlab.max() > LBL:
        return None

    core_of_row = core_of_label[y]
    perm = np.argsort(core_of_row, kind="stable")
    rows_per_core = np.bincount(core_of_row, minlength=NCORES)
    if rows_per_core.max() > PAD:
        return None
    starts = np.concatenate([[0], np.cumsum(rows_per_core)])

    n0 = np.linalg.norm(xs[0].astype(np.float64))
    u0 = (xs[0] / max(n0, 1e-12)).astype(np.float32)
    u0b = np.ascontiguousarray(np.broadcast_to(u0, (128, D)))

    in_maps = []
    for k in range(NCORES):
        rows = perm[starts[k]:starts[k + 1]]
        xsp = np.zeros((PAD, D), np.float32)
        xsp[:len(rows)] = xs[rows]
        yrel = np.zeros((PAD, 1), np.float32)
        yrel[:len(rows), 0] = local_idx[y[rows]]
        sl = slice(k * RCE, (k + 1) * RCE)
        in_maps.append({
            "yb": np.ascontiguousarray(y_bars[sl]),
            "ycls": y[sl].astype(np.float32).reshape(RCE, 1),
            "xsp": xsp,
            "yrel": yrel,
            "u0b": u0b,
        })
    return in_maps, perm, rows_per_core, starts, counts


def _host_reference(xs, y_bars, y, n_negative):
    """Exact numpy replica of the jax reference (fallback only)."""
    logits = y_bars.astype(np.float64)
    m = logits.max(axis=1, keepdims=True)
    lse = np.log(np.exp(logits - m).sum(axis=1)) + m[:, 0]
    loss1 = np.mean(lse - logits[np.arange(B), y])

    norms = np.linalg.norm(xs.astype(np.float64), axis=-1)
    cos = (xs.astype(np.float64) @ xs.T.astype(np.float64)) / np.maximum(
        norms[:, None] * norms[None, :], 1e-8)
    same = y[:, None] == y[None, :]
    triu = np.triu(np.ones((B, B), bool), k=1)
    pos = same & triu
    n_pos = pos.sum()
    loss2 = ((1.0 - cos)[pos].sum() / max(n_pos, 1)) if n_pos > 0 else 0.0
    counts = np.bincount(y, minlength=C)
    n_neg = max(int(n_negative), int((counts > 1).sum()))
    neg_flat = (~same & triu).reshape(-1)
    rank = np.cumsum(neg_flat)
    sel = neg_flat & (rank <= n_neg)
    loss3 = np.maximum(cos.reshape(-1)[sel], 0.0).sum() / n_neg
    return np.float32(loss1 + loss2 + loss3)


def kernel(xs, y_bars, y_gths, n_negative):
    global LAST_EXEC_NS
    xs = np.ascontiguousarray(np.asarray(xs, np.float32))
    y_bars = np.ascontiguousarray(np.asarray(y_bars, np.float32))
    y = np.asarray(y_gths).astype(np.int64).reshape(-1)
    n_negative = int(np.asarray(n_negative).reshape(()))
    assert xs.shape == (B, D) and y_bars.shape == (B, C) and y.shape == (B,)

    sharded = _shard(xs, y_bars, y)
    counts = np.bincount(y, minlength=C)
    n_dup = int((counts > 1).sum())
    n_neg = max(n_negative, n_dup)
    # loss3 structural requirement: first n_neg lexicographic negative pairs
    # must all be (0, j) pairs.
    neg0 = int((y[1:] != y[0]).sum())
    if sharded is None or n_neg <= 0 or neg0 < n_neg:
        return _host_reference(xs, y_bars, y, n_negative)

    in_maps, perm, rows_per_core, starts, _ = sharded
    nc = _get_program()
    res = run_bass_kernel_spmd(nc, in_maps, core_ids=list(range(NCORES)),
                               trace=bool(os.environ.get("BASS_TRACE")))
    if res.exec_time_ns is not None:
        LAST_EXEC_NS = res.exec_time_ns

    ce_all = np.concatenate([res.results[k]["ce"][:, 0] for k in range(NCORES)])
    loss1 = float(ce_all.astype(np.float64).mean())

    sum_sq = float(sum(res.results[k]["snorm"][:, 0].astype(np.float64).sum()
                       for k in range(NCORES)))
    n_pos = int((counts.astype(np.int64) * (counts.astype(np.int64) - 1) // 2).sum())
    pos_cos = (sum_sq - float(B)) / 2.0
    loss2 = (n_pos - pos_cos) / max(n_pos, 1) if n_pos > 0 else 0.0

    cos0_global = np.empty(B, np.float64)
    gathered = np.concatenate([
        res.results[k]["cos0"][:rows_per_core[k], 0] for k in range(NCORES)])
    cos0_global[perm] = gathered
    neg_j = np.nonzero(y[1:] != y[0])[0][:n_neg] + 1
    loss3 = float(np.maximum(cos0_global[neg_j], 0.0).sum()) / n_neg

    return np.asarray(loss1 + loss2 + loss3, np.float32)


# revision 11
# speedup vs baseline: 1.1221x; 1.1221x over previous
"""Trainium2 Bass kernel for nn_ContrastiveLoss (CE + pos-pair cosine + first-k
neg-pair cosine), SPMD across 8 NeuronCores.

Math used (mathematically identical to the reference, avoids the BxB cosine
matrix entirely):
  loss1 = mean_i( log(sum_c exp(yb[i,c])) - yb[i, y_i] )
  loss2: sum over same-label pairs i<j of cos_ij
           = (sum_c ||S_c||^2 - sum_i ||u_i||^2) / 2,   S_c = sum_{y_i=c} u_i,
         with u_i = x_i/||x_i||, sum_i ||u_i||^2 = B.
  loss3 needs only cos(x_0, x_j): the first n_neg (<= C = 1000) negative pairs
        in lexicographic (i<j) order all come from row i=0 whenever row 0 has
        >= n_neg differing-label partners (host-verified, else exact host
        fallback).

Sharding: labels are snake-packed onto 8 cores (125 labels/core); xs rows are
permuted so a core owns all rows of its labels -> per-core sum_c ||S_c||^2
partials are scalars (no cross-core reduction / no collectives). y_bars is
row-sharded naturally. Host glue is only O(B) index math plus summing 8 small
output vectors.
"""

import os
import numpy as np

try:
    import concourse.bass as bass  # noqa: F401
except ImportError:
    import sys

    for p in ("/opt/trn_rl_repo", "/root/.axon_site/_ro/trn_rl_repo"):
        if os.path.isdir(p):
            sys.path.insert(0, p)
            break
    import concourse.bass as bass  # noqa: F401

import concourse.tile as tile
from concourse import bacc, mybir
from concourse.bass_utils import run_bass_kernel_spmd

B, D, C, NCORES = 4096, 512, 1000, 8
RCE = B // NCORES  # 512 CE rows per core
PAD = 640          # padded loss2 row capacity per core (5 x 128)
LBL = 128          # label slots per core (1000/8 = 125 used)

F32 = mybir.dt.float32
BF16 = mybir.dt.bfloat16
ALU = mybir.AluOpType
ACTF = mybir.ActivationFunctionType

LAST_EXEC_NS = None  # filled when BASS_TRACE=1 and profiling succeeds

_PROGRAM = None  # cached (nc) build


def _build_program():
    """One SPMD program, identical on all 8 cores; all per-core variation is
    carried by the input tensors."""
    nc = bacc.Bacc("TRN2", target_bir_lowering=False, debug=False,
                   num_devices=NCORES)

    NG = RCE // 128   # 4 row-groups for CE
    NX = PAD // 128   # 5 row-groups for loss2/loss3

    yb = nc.declare_dram_parameter("yb", [RCE, C], F32, isOutput=False)
    ycls = nc.declare_dram_parameter("ycls", [RCE, 1], F32, isOutput=False)
    xsp = nc.declare_dram_parameter("xsp", [PAD, D], F32, isOutput=False)
    yrel = nc.declare_dram_parameter("yrel", [PAD, 1], F32, isOutput=False)
    u0b = nc.declare_dram_parameter("u0b", [128, D], F32, isOutput=False)

    se_o = nc.declare_dram_parameter("se_o", [RCE, 1], F32, isOutput=True)
    pk_o = nc.declare_dram_parameter("pk_o", [RCE, 1], F32, isOutput=True)
    snorm = nc.declare_dram_parameter("snorm", [128, 1], F32, isOutput=True)
    cos0 = nc.declare_dram_parameter("cos0", [PAD, 1], F32, isOutput=True)

    with tile.TileContext(nc) as tc:
        with (
            tc.tile_pool(name="const", bufs=1) as const,
            tc.tile_pool(name="io", bufs=1) as io,
            tc.tile_pool(name="trash", bufs=2) as trash,
            tc.tile_pool(name="small", bufs=4) as small,
            tc.tile_pool(name="psum", bufs=1, space="PSUM") as psum,
        ):
            iota_c = const.tile([128, C], F32)
            nc.gpsimd.iota(iota_c[:], [[1, C]], channel_multiplier=0,
                           allow_small_or_imprecise_dtypes=True)
            iota_l = const.tile([128, LBL], F32)
            nc.gpsimd.iota(iota_l[:], [[1, LBL]], channel_multiplier=0,
                           allow_small_or_imprecise_dtypes=True)

            # single 3D-pattern DMA per tensor: row (n*128 + p) lands in
            # partition p, group n (cuts SP descriptor-gen serialization)
            u0t = const.tile([128, D], F32)
            nc.sync.dma_start(u0t[:], u0b[:])
            xt = io.tile([128, NX, D], F32)
            nc.sync.dma_start(xt[:], xsp.rearrange("(n p) d -> p n d", p=128))
            yr = small.tile([128, NX], F32)
            nc.sync.dma_start(yr[:], yrel.rearrange("(n p) o -> p (n o)", p=128))
            t = io.tile([128, NG, C], F32)
            nc.sync.dma_start(t[:], yb.rearrange("(n p) c -> p n c", p=128))
            yc = small.tile([128, NG], F32)
            nc.sync.dma_start(yc[:], ycls.rearrange("(n p) o -> p (n o)", p=128))

            # ---- loss2 norms: |x|^2 per row, one Sqrt/recip for all groups
            nqs = small.tile([128, NX], F32)
            for i in range(NX):
                td = trash.tile([128, D], F32)
                nc.vector.scalar_tensor_tensor(
                    td[:], xt[:, i, :], 1.0, xt[:, i, :],
                    op0=ALU.mult, op1=ALU.mult, accum_out=nqs[:, i:i + 1])
            # floor |x|^2 at 1e-12: zero-padded rows get inv_n = 1e6 but
            # contribute x*inv_n = 0 everywhere downstream.
            nc.vector.tensor_scalar_max(nqs[:], nqs[:], 1e-12)
            nrm = small.tile([128, NX], F32)
            nc.scalar.activation(nrm[:], nqs[:], ACTF.Sqrt)
            invn = small.tile([128, NX], F32)
            nc.vector.reciprocal(invn[:], nrm[:])

            # ---- loss2 matmul + loss3 row-0 cosine -----------------------
            S = psum.tile([128, D], F32)
            c0 = small.tile([128, NX], F32)
            dt0 = small.tile([128, NX], F32)
            for i in range(NX):
                # scaled one-hot: oh[p, l] = (l == yrel_p) * inv_n_p, so the
                # matmul S = oh^T @ x accumulates already-normalized rows.
                oh = small.tile([128, LBL], F32)
                nc.vector.tensor_scalar(
                    oh[:], iota_l[:], yr[:, i:i + 1], invn[:, i:i + 1],
                    op0=ALU.is_equal, op1=ALU.mult)
                nc.tensor.matmul(S[:], oh[:], xt[:, i, :],
                                 start=(i == 0), stop=(i == NX - 1))

                # cos(x_0, x_j) = (x_j . u0) * inv_n_j for local rows
                td2 = trash.tile([128, D], F32)
                nc.vector.scalar_tensor_tensor(
                    td2[:], xt[:, i, :], 1.0, u0t[:],
                    op0=ALU.mult, op1=ALU.mult, accum_out=dt0[:, i:i + 1])
                nc.vector.tensor_mul(c0[:, i:i + 1], dt0[:, i:i + 1],
                                     invn[:, i:i + 1])
            nc.sync.dma_start(cos0.rearrange("(n p) o -> p (n o)", p=128), c0[:])

            # ---- loss1: exp+rowsum (one ACT pass/group) + label gather ---
            ses = small.tile([128, NG], F32)
            pks = small.tile([128, NG], F32)
            for i in range(NG):
                ex = trash.tile([128, C], BF16)
                nc.scalar.activation(ex[:], t[:, i, :], ACTF.Exp,
                                     accum_out=ses[:, i:i + 1])
                tt = trash.tile([128, C], F32)
                nc.vector.scalar_tensor_tensor(
                    tt[:], iota_c[:], yc[:, i:i + 1], t[:, i, :],
                    op0=ALU.is_equal, op1=ALU.mult, accum_out=pks[:, i:i + 1])
            nc.sync.dma_start(se_o.rearrange("(n p) o -> p (n o)", p=128), ses[:])
            nc.sync.dma_start(pk_o.rearrange("(n p) o -> p (n o)", p=128), pks[:])

            # ---- ||S_c||^2 (DVE may read only one PSUM operand) ----------
            Ssb = io.tile([128, D], F32)
            nc.vector.tensor_copy(Ssb[:], S[:])
            sn = small.tile([128, 1], F32)
            td3 = trash.tile([128, D], F32)
            nc.vector.scalar_tensor_tensor(
                td3[:], Ssb[:], 1.0, Ssb[:],
                op0=ALU.mult, op1=ALU.mult, accum_out=sn[:])
            nc.sync.dma_start(snorm[:], sn[:])

    nc.compile()
    return nc


def _get_program():
    global _PROGRAM
    if _PROGRAM is None:
        _PROGRAM = _build_program()
    return _PROGRAM


def _shard(xs, y_bars, y):
    """Snake-pack labels onto cores; permute xs rows into per-core blocks.

    Returns in_maps plus the bookkeeping needed to reassemble outputs, or
    None if capacity assumptions fail (handled by exact host fallback).
    """
    counts = np.bincount(y, minlength=C).astype(np.int64)
    order = np.argsort(-counts, kind="stable")  # labels by count desc
    core_of_label = np.empty(C, np.int32)
    local_idx = np.empty(C, np.int32)
    nlab = np.zeros(NCORES, np.int32)
    for r in range(0, C, NCORES):
        blk = order[r:r + NCORES]
        cores = range(NCORES) if (r // NCORES) % 2 == 0 else range(NCORES - 1, -1, -1)
        for lab, core in zip(blk, cores):
            core_of_label[lab] = core
            local_idx[lab] = nlab[core]
            nlab[core] += 1
    if nlab.max() > LBL:
        return None

    core_of_row = core_of_label[y]
    perm = np.argsort(core_of_row, kind="stable")
    rows_per_core = np.bincount(core_of_row, minlength=NCORES)
    if rows_per_core.max() > PAD:
        return None
    starts = np.concatenate([[0], np.cumsum(rows_per_core)])

    n0 = np.linalg.norm(xs[0].astype(np.float64))
    u0 = (xs[0] / max(n0, 1e-12)).astype(np.float32)
    u0b = np.ascontiguousarray(np.broadcast_to(u0, (128, D)))

    in_maps = []
    for k in range(NCORES):
        rows = perm[starts[k]:starts[k + 1]]
        xsp = np.zeros((PAD, D), np.float32)
        xsp[:len(rows)] = xs[rows]
        yrel = np.zeros((PAD, 1), np.float32)
        yrel[:len(rows), 0] = local_idx[y[rows]]
        sl = slice(k * RCE, (k + 1) * RCE)
        in_maps.append({
            "yb": np.ascontiguousarray(y_bars[sl]),
            "ycls": y[sl].astype(np.float32).reshape(RCE, 1),
            "xsp": xsp,
            "yrel": yrel,
            "u0b": u0b,
        })
    return in_maps, perm, rows_per_core, starts, counts


def _host_reference(xs, y_bars, y, n_negative):
    """Exact numpy replica of the jax reference (fallback only)."""
    logits = y_bars.astype(np.float64)
    m = logits.max(axis=1, keepdims=True)
    lse = np.log(np.exp(logits - m).sum(axis=1)) + m[:, 0]
    loss1 = np.mean(lse - logits[np.arange(B), y])

    norms = np.linalg.norm(xs.astype(np.float64), axis=-1)
    cos = (xs.astype(np.float64) @ xs.T.astype(np.float64)) / np.maximum(
        norms[:, None] * norms[None, :], 1e-8)
    same = y[:, None] == y[None, :]
    triu = np.triu(np.ones((B, B), bool), k=1)
    pos = same & triu
    n_pos = pos.sum()
    loss2 = ((1.0 - cos)[pos].sum() / max(n_pos, 1)) if n_pos > 0 else 0.0
    counts = np.bincount(y, minlength=C)
    n_neg = max(int(n_negative), int((counts > 1).sum()))
    neg_flat = (~same & triu).reshape(-1)
    rank = np.cumsum(neg_flat)
    sel = neg_flat & (rank <= n_neg)
    loss3 = np.maximum(cos.reshape(-1)[sel], 0.0).sum() / n_neg
    return np.float32(loss1 + loss2 + loss3)


def kernel(xs, y_bars, y_gths, n_negative):
    global LAST_EXEC_NS
    xs = np.ascontiguousarray(np.asarray(xs, np.float32))
    y_bars = np.ascontiguousarray(np.asarray(y_bars, np.float32))
    y = np.asarray(y_gths).astype(np.int64).reshape(-1)
    n_negative = int(np.asarray(n_negative).reshape(()))
    assert xs.shape == (B, D) and y_bars.shape == (B, C) and y.shape == (B,)

    sharded = _shard(xs, y_bars, y)
    counts = np.bincount(y, minlength=C)
    n_dup = int((counts > 1).sum())
    n_neg = max(n_negative, n_dup)
    # loss3 structural requirement: first n_neg lexicographic negative pairs
    # must all be (0, j) pairs.
    neg0 = int((y[1:] != y[0]).sum())
    if sharded is None or n_neg <= 0 or neg0 < n_neg:
        return _host_reference(xs, y_bars, y, n_negative)

    in_maps, perm, rows_per_core, starts, _ = sharded
    nc = _get_program()
    res = run_bass_kernel_spmd(nc, in_maps, core_ids=list(range(NCORES)),
                               trace=bool(os.environ.get("BASS_TRACE")))
    if res.exec_time_ns is not None:
        LAST_EXEC_NS = res.exec_time_ns

    se_all = np.concatenate([res.results[k]["se_o"][:, 0] for k in range(NCORES)])
    pk_all = np.concatenate([res.results[k]["pk_o"][:, 0] for k in range(NCORES)])
    loss1 = float((np.log(se_all.astype(np.float64)) - pk_all).mean())

    sum_sq = float(sum(res.results[k]["snorm"][:, 0].astype(np.float64).sum()
                       for k in range(NCORES)))
    n_pos = int((counts.astype(np.int64) * (counts.astype(np.int64) - 1) // 2).sum())
    pos_cos = (sum_sq - float(B)) / 2.0
    loss2 = (n_pos - pos_cos) / max(n_pos, 1) if n_pos > 0 else 0.0

    cos0_global = np.empty(B, np.float64)
    gathered = np.concatenate([
        res.results[k]["cos0"][:rows_per_core[k], 0] for k in range(NCORES)])
    cos0_global[perm] = gathered
    neg_j = np.nonzero(y[1:] != y[0])[0][:n_neg] + 1
    loss3 = float(np.maximum(cos0_global[neg_j], 0.0).sum()) / n_neg

    return np.asarray(loss1 + loss2 + loss3, np.float32)


# revision 14
# speedup vs baseline: 1.8885x; 1.6830x over previous
"""Trainium2 Bass kernel for nn_ContrastiveLoss (CE + pos-pair cosine + first-k
neg-pair cosine), SPMD across 8 NeuronCores.

Math used (mathematically identical to the reference, avoids the BxB cosine
matrix entirely):
  loss1 = mean_i( log(sum_c exp(yb[i,c])) - yb[i, y_i] )
  loss2: sum over same-label pairs i<j of cos_ij
           = (sum_c ||S_c||^2 - sum_i ||u_i||^2) / 2,   S_c = sum_{y_i=c} u_i,
         with u_i = x_i/||x_i||, sum_i ||u_i||^2 = B.
  loss3 needs only cos(x_0, x_j): the first n_neg (<= C = 1000) negative pairs
        in lexicographic (i<j) order all come from row i=0 whenever row 0 has
        >= n_neg differing-label partners (host-verified, else exact host
        fallback).

Sharding: labels are snake-packed onto 8 cores (125 labels/core); xs rows are
permuted so a core owns all rows of its labels -> per-core sum_c ||S_c||^2
partials are scalars (no cross-core reduction / no collectives). y_bars is
row-sharded naturally. Host glue is only O(B) index math plus summing 8 small
output vectors.
"""

import os
import numpy as np

try:
    import concourse.bass as bass  # noqa: F401
except ImportError:
    import sys

    for p in ("/opt/trn_rl_repo", "/root/.axon_site/_ro/trn_rl_repo"):
        if os.path.isdir(p):
            sys.path.insert(0, p)
            break
    import concourse.bass as bass  # noqa: F401

import concourse.tile as tile
from concourse import bacc, mybir
from concourse.bass_utils import run_bass_kernel_spmd

B, D, C, NCORES = 4096, 512, 1000, 8
RCE = B // NCORES  # 512 CE rows per core
PAD = 640          # padded loss2 row capacity per core (5 x 128)
LBL = 128          # label slots per core (1000/8 = 125 used)

F32 = mybir.dt.float32
BF16 = mybir.dt.bfloat16
ALU = mybir.AluOpType
ACTF = mybir.ActivationFunctionType

LAST_EXEC_NS = None  # filled when BASS_TRACE=1 and profiling succeeds

_PROGRAM = None  # cached (nc) build


def _build_program():
    """One SPMD program, identical on all 8 cores; all per-core variation is
    carried by the input tensors."""
    nc = bacc.Bacc("TRN2", target_bir_lowering=False, debug=False,
                   num_devices=NCORES)

    NG = RCE // 128   # 4 row-groups for CE
    NX = PAD // 128   # 5 row-groups for loss2/loss3

    # packed per-core output columns: se[0:NG], pk[NG:2NG], cos0[2NG:2NG+NX],
    # snorm[2NG+NX]
    OCOLS = 2 * NG + NX + 1

    yb = nc.declare_dram_parameter("yb", [RCE, C], F32, isOutput=False)
    ycls = nc.declare_dram_parameter("ycls", [128, NG], F32, isOutput=False)
    xsp = nc.declare_dram_parameter("xsp", [PAD, D], F32, isOutput=False)
    yrel = nc.declare_dram_parameter("yrel", [128, NX], F32, isOutput=False)
    u0b = nc.declare_dram_parameter("u0b", [128, D], F32, isOutput=False)
    outp = nc.declare_dram_parameter("outp", [128, OCOLS], F32, isOutput=True)

    with tile.TileContext(nc) as tc:
        with (
            tc.tile_pool(name="const", bufs=1) as const,
            tc.tile_pool(name="io", bufs=1) as io,
            tc.tile_pool(name="trash", bufs=2) as trash,
            tc.tile_pool(name="small", bufs=4) as small,
            tc.tile_pool(name="psum", bufs=1, space="PSUM") as psum,
        ):
            iota_c = const.tile([128, C], F32)
            nc.gpsimd.iota(iota_c[:], [[1, C]], channel_multiplier=0,
                           allow_small_or_imprecise_dtypes=True)
            iota_l = const.tile([128, LBL], F32)
            nc.gpsimd.iota(iota_l[:], [[1, LBL]], channel_multiplier=0,
                           allow_small_or_imprecise_dtypes=True)
            op = small.tile([128, OCOLS], F32)

            # 3D-pattern chunked DMAs: row (n*128 + p) lands in partition p,
            # group n. Two chunks per big tensor so group-0 compute overlaps
            # the remaining transfer; small tensors are partition-major on
            # the host side (contiguous per-partition runs, no 4B scatter).
            xsp_r = xsp.rearrange("(n p) d -> p n d", p=128)
            yb_r = yb.rearrange("(n p) c -> p n c", p=128)
            u0t = const.tile([128, D], F32)
            nc.sync.dma_start(u0t[:], u0b[:])
            yr = small.tile([128, NX], F32)
            nc.sync.dma_start(yr[:], yrel[:])
            yc = small.tile([128, NG], F32)
            nc.sync.dma_start(yc[:], ycls[:])
            xt = io.tile([128, NX, D], F32)
            nc.sync.dma_start(xt[:, 0:3, :], xsp_r[:, 0:3, :])
            t = io.tile([128, NG, C], F32)
            nc.sync.dma_start(t[:, 0:2, :], yb_r[:, 0:2, :])
            nc.sync.dma_start(xt[:, 3:NX, :], xsp_r[:, 3:NX, :])
            nc.sync.dma_start(t[:, 2:NG, :], yb_r[:, 2:NG, :])

            # ---- loss2 norms: |x|^2 per row, one Sqrt/recip for all groups
            nqs = small.tile([128, NX], F32)
            for i in range(NX):
                td = trash.tile([128, D], F32)
                nc.vector.scalar_tensor_tensor(
                    td[:], xt[:, i, :], 1.0, xt[:, i, :],
                    op0=ALU.mult, op1=ALU.mult, accum_out=nqs[:, i:i + 1])
            # floor |x|^2 at 1e-12: zero-padded rows get inv_n = 1e6 but
            # contribute x*inv_n = 0 everywhere downstream.
            nc.vector.tensor_scalar_max(nqs[:], nqs[:], 1e-12)
            nrm = small.tile([128, NX], F32)
            nc.scalar.activation(nrm[:], nqs[:], ACTF.Sqrt)
            invn = small.tile([128, NX], F32)
            nc.vector.reciprocal(invn[:], nrm[:])

            # ---- loss2 matmul + loss3 row-0 cosine -----------------------
            S = psum.tile([128, D], F32)
            dt0 = small.tile([128, NX], F32)
            for i in range(NX):
                # scaled one-hot: oh[p, l] = (l == yrel_p) * inv_n_p, so the
                # matmul S = oh^T @ x accumulates already-normalized rows.
                oh = small.tile([128, LBL], F32)
                nc.vector.tensor_scalar(
                    oh[:], iota_l[:], yr[:, i:i + 1], invn[:, i:i + 1],
                    op0=ALU.is_equal, op1=ALU.mult)
                nc.tensor.matmul(S[:], oh[:], xt[:, i, :],
                                 start=(i == 0), stop=(i == NX - 1))

                # cos(x_0, x_j) = (x_j . u0) * inv_n_j for local rows
                td2 = trash.tile([128, D], F32)
                nc.vector.scalar_tensor_tensor(
                    td2[:], xt[:, i, :], 1.0, u0t[:],
                    op0=ALU.mult, op1=ALU.mult, accum_out=dt0[:, i:i + 1])
                nc.vector.tensor_mul(op[:, 2 * NG + i:2 * NG + i + 1],
                                     dt0[:, i:i + 1], invn[:, i:i + 1])

            # ---- loss1: exp+rowsum (one ACT pass/group) + label gather ---
            for i in range(NG):
                ex = trash.tile([128, C], BF16)
                nc.scalar.activation(ex[:], t[:, i, :], ACTF.Exp,
                                     accum_out=op[:, i:i + 1])
                tt = trash.tile([128, C], F32)
                nc.vector.scalar_tensor_tensor(
                    tt[:], iota_c[:], yc[:, i:i + 1], t[:, i, :],
                    op0=ALU.is_equal, op1=ALU.mult,
                    accum_out=op[:, NG + i:NG + i + 1])

            # ---- ||S_c||^2 (DVE may read only one PSUM operand) ----------
            Ssb = io.tile([128, D], F32)
            nc.vector.tensor_copy(Ssb[:], S[:])
            td3 = trash.tile([128, D], F32)
            nc.vector.scalar_tensor_tensor(
                td3[:], Ssb[:], 1.0, Ssb[:],
                op0=ALU.mult, op1=ALU.mult,
                accum_out=op[:, OCOLS - 1:OCOLS])

            nc.sync.dma_start(outp[:], op[:])

    nc.compile()
    return nc


def _get_program():
    global _PROGRAM
    if _PROGRAM is None:
        _PROGRAM = _build_program()
    return _PROGRAM


def _shard(xs, y_bars, y):
    """Snake-pack labels onto cores; permute xs rows into per-core blocks.

    Returns in_maps plus the bookkeeping needed to reassemble outputs, or
    None if capacity assumptions fail (handled by exact host fallback).
    """
    counts = np.bincount(y, minlength=C).astype(np.int64)
    order = np.argsort(-counts, kind="stable")  # labels by count desc
    core_of_label = np.empty(C, np.int32)
    local_idx = np.empty(C, np.int32)
    nlab = np.zeros(NCORES, np.int32)
    for r in range(0, C, NCORES):
        blk = order[r:r + NCORES]
        cores = range(NCORES) if (r // NCORES) % 2 == 0 else range(NCORES - 1, -1, -1)
        for lab, core in zip(blk, cores):
            core_of_label[lab] = core
            local_idx[lab] = nlab[core]
            nlab[core] += 1
    if nlab.max() > LBL:
        return None

    core_of_row = core_of_label[y]
    perm = np.argsort(core_of_row, kind="stable")
    rows_per_core = np.bincount(core_of_row, minlength=NCORES)
    if rows_per_core.max() > PAD:
        return None
    starts = np.concatenate([[0], np.cumsum(rows_per_core)])

    n0 = np.linalg.norm(xs[0].astype(np.float64))
    u0 = (xs[0] / max(n0, 1e-12)).astype(np.float32)
    u0b = np.ascontiguousarray(np.broadcast_to(u0, (128, D)))

    in_maps = []
    for k in range(NCORES):
        rows = perm[starts[k]:starts[k + 1]]
        xsp = np.zeros((PAD, D), np.float32)
        xsp[:len(rows)] = xs[rows]
        yrel = np.zeros(PAD, np.float32)
        yrel[:len(rows)] = local_idx[y[rows]]
        sl = slice(k * RCE, (k + 1) * RCE)
        # ycls/yrel partition-major: [p, n] = value for row n*128 + p
        in_maps.append({
            "yb": np.ascontiguousarray(y_bars[sl]),
            "ycls": np.ascontiguousarray(
                y[sl].astype(np.float32).reshape(-1, 128).T),
            "xsp": xsp,
            "yrel": np.ascontiguousarray(yrel.reshape(-1, 128).T),
            "u0b": u0b,
        })
    return in_maps, perm, rows_per_core, starts, counts


def _host_reference(xs, y_bars, y, n_negative):
    """Exact numpy replica of the jax reference (fallback only)."""
    logits = y_bars.astype(np.float64)
    m = logits.max(axis=1, keepdims=True)
    lse = np.log(np.exp(logits - m).sum(axis=1)) + m[:, 0]
    loss1 = np.mean(lse - logits[np.arange(B), y])

    norms = np.linalg.norm(xs.astype(np.float64), axis=-1)
    cos = (xs.astype(np.float64) @ xs.T.astype(np.float64)) / np.maximum(
        norms[:, None] * norms[None, :], 1e-8)
    same = y[:, None] == y[None, :]
    triu = np.triu(np.ones((B, B), bool), k=1)
    pos = same & triu
    n_pos = pos.sum()
    loss2 = ((1.0 - cos)[pos].sum() / max(n_pos, 1)) if n_pos > 0 else 0.0
    counts = np.bincount(y, minlength=C)
    n_neg = max(int(n_negative), int((counts > 1).sum()))
    neg_flat = (~same & triu).reshape(-1)
    rank = np.cumsum(neg_flat)
    sel = neg_flat & (rank <= n_neg)
    loss3 = np.maximum(cos.reshape(-1)[sel], 0.0).sum() / n_neg
    return np.float32(loss1 + loss2 + loss3)


def kernel(xs, y_bars, y_gths, n_negative):
    global LAST_EXEC_NS
    xs = np.ascontiguousarray(np.asarray(xs, np.float32))
    y_bars = np.ascontiguousarray(np.asarray(y_bars, np.float32))
    y = np.asarray(y_gths).astype(np.int64).reshape(-1)
    n_negative = int(np.asarray(n_negative).reshape(()))
    assert xs.shape == (B, D) and y_bars.shape == (B, C) and y.shape == (B,)

    sharded = _shard(xs, y_bars, y)
    counts = np.bincount(y, minlength=C)
    n_dup = int((counts > 1).sum())
    n_neg = max(n_negative, n_dup)
    # loss3 structural requirement: first n_neg lexicographic negative pairs
    # must all be (0, j) pairs.
    neg0 = int((y[1:] != y[0]).sum())
    if sharded is None or n_neg <= 0 or neg0 < n_neg:
        return _host_reference(xs, y_bars, y, n_negative)

    in_maps, perm, rows_per_core, starts, _ = sharded
    nc = _get_program()
    res = run_bass_kernel_spmd(nc, in_maps, core_ids=list(range(NCORES)),
                               trace=bool(os.environ.get("BASS_TRACE")))
    if res.exec_time_ns is not None:
        LAST_EXEC_NS = res.exec_time_ns

    NG, NX = RCE // 128, PAD // 128
    outs = [res.results[k]["outp"] for k in range(NCORES)]
    # packed columns: [p, n] holds row n*128 + p of the per-core vector
    se_all = np.concatenate([o[:, 0:NG].T.reshape(-1) for o in outs])
    pk_all = np.concatenate([o[:, NG:2 * NG].T.reshape(-1) for o in outs])
    loss1 = float((np.log(se_all.astype(np.float64)) - pk_all).mean())

    sum_sq = float(sum(o[:, -1].astype(np.float64).sum() for o in outs))
    n_pos = int((counts.astype(np.int64) * (counts.astype(np.int64) - 1) // 2).sum())
    pos_cos = (sum_sq - float(B)) / 2.0
    loss2 = (n_pos - pos_cos) / max(n_pos, 1) if n_pos > 0 else 0.0

    cos0_global = np.empty(B, np.float64)
    gathered = np.concatenate([
        outs[k][:, 2 * NG:2 * NG + NX].T.reshape(-1)[:rows_per_core[k]]
        for k in range(NCORES)])
    cos0_global[perm] = gathered
    neg_j = np.nonzero(y[1:] != y[0])[0][:n_neg] + 1
    loss3 = float(np.maximum(cos0_global[neg_j], 0.0).sum()) / n_neg

    return np.asarray(loss1 + loss2 + loss3, np.float32)


# revision 15
# speedup vs baseline: 2.1216x; 1.1235x over previous
"""Trainium2 Bass kernel for nn_ContrastiveLoss (CE + pos-pair cosine + first-k
neg-pair cosine), SPMD across 8 NeuronCores.

Math used (mathematically identical to the reference, avoids the BxB cosine
matrix entirely):
  loss1 = mean_i( log(sum_c exp(yb[i,c])) - yb[i, y_i] )
  loss2: sum over same-label pairs i<j of cos_ij
           = (sum_c ||S_c||^2 - sum_i ||u_i||^2) / 2,   S_c = sum_{y_i=c} u_i,
         with u_i = x_i/||x_i||, sum_i ||u_i||^2 = B.
  loss3 needs only cos(x_0, x_j): the first n_neg (<= C = 1000) negative pairs
        in lexicographic (i<j) order all come from row i=0 whenever row 0 has
        >= n_neg differing-label partners (host-verified, else exact host
        fallback).

Sharding: labels are snake-packed onto 8 cores (125 labels/core); xs rows are
permuted so a core owns all rows of its labels -> per-core sum_c ||S_c||^2
partials are scalars (no cross-core reduction / no collectives). y_bars is
row-sharded naturally. Host glue is only O(B) index math plus summing 8 small
output vectors.
"""

import os
import numpy as np

try:
    import concourse.bass as bass  # noqa: F401
except ImportError:
    import sys

    for p in ("/opt/trn_rl_repo", "/root/.axon_site/_ro/trn_rl_repo"):
        if os.path.isdir(p):
            sys.path.insert(0, p)
            break
    import concourse.bass as bass  # noqa: F401

import concourse.tile as tile
from concourse import bacc, mybir
from concourse.bass_utils import run_bass_kernel_spmd

B, D, C, NCORES = 4096, 512, 1000, 8
RCE = B // NCORES  # 512 CE rows per core
PAD = 640          # padded loss2 row capacity per core (5 x 128)
LBL = 128          # label slots per core (1000/8 = 125 used)

F32 = mybir.dt.float32
BF16 = mybir.dt.bfloat16
ALU = mybir.AluOpType
ACTF = mybir.ActivationFunctionType

LAST_EXEC_NS = None  # filled when BASS_TRACE=1 and profiling succeeds

_PROGRAM = None  # cached (nc) build


def _build_program():
    """One SPMD program, identical on all 8 cores; all per-core variation is
    carried by the input tensors."""
    nc = bacc.Bacc("TRN2", target_bir_lowering=False, debug=False,
                   num_devices=NCORES)

    NG = RCE // 128   # 4 row-groups for CE
    NX = PAD // 128   # 5 row-groups for loss2/loss3

    # packed per-core output columns: se[0:NG], pk[NG:2NG], cos0[2NG:2NG+NX],
    # snorm[2NG+NX]
    OCOLS = 2 * NG + NX + 1

    yb = nc.declare_dram_parameter("yb", [RCE, C], F32, isOutput=False)
    ycls = nc.declare_dram_parameter("ycls", [128, NG], F32, isOutput=False)
    xsp = nc.declare_dram_parameter("xsp", [PAD, D], F32, isOutput=False)
    yrel = nc.declare_dram_parameter("yrel", [128, NX], F32, isOutput=False)
    u0b = nc.declare_dram_parameter("u0b", [128, D], F32, isOutput=False)
    outp = nc.declare_dram_parameter("outp", [128, OCOLS], F32, isOutput=True)

    with tile.TileContext(nc) as tc:
        with (
            tc.tile_pool(name="const", bufs=1) as const,
            tc.tile_pool(name="io", bufs=1) as io,
            tc.tile_pool(name="trash", bufs=2) as trash,
            tc.tile_pool(name="small", bufs=4) as small,
            tc.tile_pool(name="psum", bufs=1, space="PSUM") as psum,
        ):
            iota_c = const.tile([128, C], F32)
            nc.gpsimd.iota(iota_c[:], [[1, C]], channel_multiplier=0,
                           allow_small_or_imprecise_dtypes=True)
            iota_l = const.tile([128, LBL], F32)
            nc.gpsimd.iota(iota_l[:], [[1, LBL]], channel_multiplier=0,
                           allow_small_or_imprecise_dtypes=True)
            op = small.tile([128, OCOLS], F32)

            # 3D-pattern chunked DMAs: row (n*128 + p) lands in partition p,
            # group n. Two chunks per big tensor so group-0 compute overlaps
            # the remaining transfer; small tensors are partition-major on
            # the host side (contiguous per-partition runs, no 4B scatter).
            xsp_r = xsp.rearrange("(n p) d -> p n d", p=128)
            yb_r = yb.rearrange("(n p) c -> p n c", p=128)
            u0t = const.tile([128, D], F32)
            nc.sync.dma_start(u0t[:], u0b[:])
            yr = small.tile([128, NX], F32)
            nc.sync.dma_start(yr[:], yrel[:])
            yc = small.tile([128, NG], F32)
            nc.sync.dma_start(yc[:], ycls[:])
            # xsp fully first: the loss2 chain (norms -> sqrt -> one-hot ->
            # matmul) is the longest dependency chain, and putting Sqrt
            # before any Exp keeps ACT at 2 table loads.
            xt = io.tile([128, NX, D], F32)
            nc.sync.dma_start(xt[:, 0:3, :], xsp_r[:, 0:3, :])
            nc.sync.dma_start(xt[:, 3:NX, :], xsp_r[:, 3:NX, :])
            t = io.tile([128, NG, C], F32)
            nc.sync.dma_start(t[:, 0:2, :], yb_r[:, 0:2, :])
            nc.sync.dma_start(t[:, 2:NG, :], yb_r[:, 2:NG, :])

            # ---- loss2 norms: |x|^2 per row, one Sqrt/recip for all groups
            nqs = small.tile([128, NX], F32)
            for i in range(NX):
                td = trash.tile([128, D], F32)
                nc.vector.scalar_tensor_tensor(
                    td[:], xt[:, i, :], 1.0, xt[:, i, :],
                    op0=ALU.mult, op1=ALU.mult, accum_out=nqs[:, i:i + 1])
            # floor |x|^2 at 1e-12: zero-padded rows get inv_n = 1e6 but
            # contribute x*inv_n = 0 everywhere downstream.
            nc.vector.tensor_scalar_max(nqs[:], nqs[:], 1e-12)
            nrm = small.tile([128, NX], F32)
            nc.scalar.activation(nrm[:], nqs[:], ACTF.Sqrt)
            invn = small.tile([128, NX], F32)
            nc.vector.reciprocal(invn[:], nrm[:])

            # ---- loss2 matmul + loss3 row-0 cosine -----------------------
            S = psum.tile([128, D], F32)
            dt0 = small.tile([128, NX], F32)
            for i in range(NX):
                # scaled one-hot: oh[p, l] = (l == yrel_p) * inv_n_p, so the
                # matmul S = oh^T @ x accumulates already-normalized rows.
                oh = small.tile([128, LBL], F32)
                nc.vector.tensor_scalar(
                    oh[:], iota_l[:], yr[:, i:i + 1], invn[:, i:i + 1],
                    op0=ALU.is_equal, op1=ALU.mult)
                nc.tensor.matmul(S[:], oh[:], xt[:, i, :],
                                 start=(i == 0), stop=(i == NX - 1))

                # cos(x_0, x_j) = (x_j . u0) * inv_n_j for local rows
                td2 = trash.tile([128, D], F32)
                nc.vector.scalar_tensor_tensor(
                    td2[:], xt[:, i, :], 1.0, u0t[:],
                    op0=ALU.mult, op1=ALU.mult, accum_out=dt0[:, i:i + 1])
                nc.vector.tensor_mul(op[:, 2 * NG + i:2 * NG + i + 1],
                                     dt0[:, i:i + 1], invn[:, i:i + 1])

            # ---- loss1: exp+rowsum (one ACT pass/group) + label gather ---
            for i in range(NG):
                ex = trash.tile([128, C], BF16)
                nc.scalar.activation(ex[:], t[:, i, :], ACTF.Exp,
                                     accum_out=op[:, i:i + 1])
                tt = trash.tile([128, C], F32)
                nc.vector.scalar_tensor_tensor(
                    tt[:], iota_c[:], yc[:, i:i + 1], t[:, i, :],
                    op0=ALU.is_equal, op1=ALU.mult,
                    accum_out=op[:, NG + i:NG + i + 1])

            # ---- ||S_c||^2 (DVE may read only one PSUM operand) ----------
            Ssb = io.tile([128, D], F32)
            nc.vector.tensor_copy(Ssb[:], S[:])
            td3 = trash.tile([128, D], F32)
            nc.vector.scalar_tensor_tensor(
                td3[:], Ssb[:], 1.0, Ssb[:],
                op0=ALU.mult, op1=ALU.mult,
                accum_out=op[:, OCOLS - 1:OCOLS])

            nc.sync.dma_start(outp[:], op[:])

    nc.compile()
    return nc


def _get_program():
    global _PROGRAM
    if _PROGRAM is None:
        _PROGRAM = _build_program()
    return _PROGRAM


def _shard(xs, y_bars, y):
    """Snake-pack labels onto cores; permute xs rows into per-core blocks.

    Returns in_maps plus the bookkeeping needed to reassemble outputs, or
    None if capacity assumptions fail (handled by exact host fallback).
    """
    counts = np.bincount(y, minlength=C).astype(np.int64)
    order = np.argsort(-counts, kind="stable")  # labels by count desc
    core_of_label = np.empty(C, np.int32)
    local_idx = np.empty(C, np.int32)
    nlab = np.zeros(NCORES, np.int32)
    for r in range(0, C, NCORES):
        blk = order[r:r + NCORES]
        cores = range(NCORES) if (r // NCORES) % 2 == 0 else range(NCORES - 1, -1, -1)
        for lab, core in zip(blk, cores):
            core_of_label[lab] = core
            local_idx[lab] = nlab[core]
            nlab[core] += 1
    if nlab.max() > LBL:
        return None

    core_of_row = core_of_label[y]
    perm = np.argsort(core_of_row, kind="stable")
    rows_per_core = np.bincount(core_of_row, minlength=NCORES)
    if rows_per_core.max() > PAD:
        return None
    starts = np.concatenate([[0], np.cumsum(rows_per_core)])

    n0 = np.linalg.norm(xs[0].astype(np.float64))
    u0 = (xs[0] / max(n0, 1e-12)).astype(np.float32)
    u0b = np.ascontiguousarray(np.broadcast_to(u0, (128, D)))

    in_maps = []
    for k in range(NCORES):
        rows = perm[starts[k]:starts[k + 1]]
        xsp = np.zeros((PAD, D), np.float32)
        xsp[:len(rows)] = xs[rows]
        yrel = np.zeros(PAD, np.float32)
        yrel[:len(rows)] = local_idx[y[rows]]
        sl = slice(k * RCE, (k + 1) * RCE)
        # ycls/yrel partition-major: [p, n] = value for row n*128 + p
        in_maps.append({
            "yb": np.ascontiguousarray(y_bars[sl]),
            "ycls": np.ascontiguousarray(
                y[sl].astype(np.float32).reshape(-1, 128).T),
            "xsp": xsp,
            "yrel": np.ascontiguousarray(yrel.reshape(-1, 128).T),
            "u0b": u0b,
        })
    return in_maps, perm, rows_per_core, starts, counts


def _host_reference(xs, y_bars, y, n_negative):
    """Exact numpy replica of the jax reference (fallback only)."""
    logits = y_bars.astype(np.float64)
    m = logits.max(axis=1, keepdims=True)
    lse = np.log(np.exp(logits - m).sum(axis=1)) + m[:, 0]
    loss1 = np.mean(lse - logits[np.arange(B), y])

    norms = np.linalg.norm(xs.astype(np.float64), axis=-1)
    cos = (xs.astype(np.float64) @ xs.T.astype(np.float64)) / np.maximum(
        norms[:, None] * norms[None, :], 1e-8)
    same = y[:, None] == y[None, :]
    triu = np.triu(np.ones((B, B), bool), k=1)
    pos = same & triu
    n_pos = pos.sum()
    loss2 = ((1.0 - cos)[pos].sum() / max(n_pos, 1)) if n_pos > 0 else 0.0
    counts = np.bincount(y, minlength=C)
    n_neg = max(int(n_negative), int((counts > 1).sum()))
    neg_flat = (~same & triu).reshape(-1)
    rank = np.cumsum(neg_flat)
    sel = neg_flat & (rank <= n_neg)
    loss3 = np.maximum(cos.reshape(-1)[sel], 0.0).sum() / n_neg
    return np.float32(loss1 + loss2 + loss3)


def kernel(xs, y_bars, y_gths, n_negative):
    global LAST_EXEC_NS
    xs = np.ascontiguousarray(np.asarray(xs, np.float32))
    y_bars = np.ascontiguousarray(np.asarray(y_bars, np.float32))
    y = np.asarray(y_gths).astype(np.int64).reshape(-1)
    n_negative = int(np.asarray(n_negative).reshape(()))
    assert xs.shape == (B, D) and y_bars.shape == (B, C) and y.shape == (B,)

    sharded = _shard(xs, y_bars, y)
    counts = np.bincount(y, minlength=C)
    n_dup = int((counts > 1).sum())
    n_neg = max(n_negative, n_dup)
    # loss3 structural requirement: first n_neg lexicographic negative pairs
    # must all be (0, j) pairs.
    neg0 = int((y[1:] != y[0]).sum())
    if sharded is None or n_neg <= 0 or neg0 < n_neg:
        return _host_reference(xs, y_bars, y, n_negative)

    in_maps, perm, rows_per_core, starts, _ = sharded
    nc = _get_program()
    res = run_bass_kernel_spmd(nc, in_maps, core_ids=list(range(NCORES)),
                               trace=bool(os.environ.get("BASS_TRACE")))
    if res.exec_time_ns is not None:
        LAST_EXEC_NS = res.exec_time_ns

    NG, NX = RCE // 128, PAD // 128
    outs = [res.results[k]["outp"] for k in range(NCORES)]
    # packed columns: [p, n] holds row n*128 + p of the per-core vector
    se_all = np.concatenate([o[:, 0:NG].T.reshape(-1) for o in outs])
    pk_all = np.concatenate([o[:, NG:2 * NG].T.reshape(-1) for o in outs])
    loss1 = float((np.log(se_all.astype(np.float64)) - pk_all).mean())

    sum_sq = float(sum(o[:, -1].astype(np.float64).sum() for o in outs))
    n_pos = int((counts.astype(np.int64) * (counts.astype(np.int64) - 1) // 2).sum())
    pos_cos = (sum_sq - float(B)) / 2.0
    loss2 = (n_pos - pos_cos) / max(n_pos, 1) if n_pos > 0 else 0.0

    cos0_global = np.empty(B, np.float64)
    gathered = np.concatenate([
        outs[k][:, 2 * NG:2 * NG + NX].T.reshape(-1)[:rows_per_core[k]]
        for k in range(NCORES)])
    cos0_global[perm] = gathered
    neg_j = np.nonzero(y[1:] != y[0])[0][:n_neg] + 1
    loss3 = float(np.maximum(cos0_global[neg_j], 0.0).sum()) / n_neg

    return np.asarray(loss1 + loss2 + loss3, np.float32)


# revision 18
# speedup vs baseline: 2.2227x; 1.0476x over previous
"""Trainium2 Bass kernel for nn_ContrastiveLoss (CE + pos-pair cosine + first-k
neg-pair cosine), SPMD across 8 NeuronCores.

Math used (mathematically identical to the reference, avoids the BxB cosine
matrix entirely):
  loss1 = mean_i( log(sum_c exp(yb[i,c])) - yb[i, y_i] )
  loss2: sum over same-label pairs i<j of cos_ij
           = (sum_c ||S_c||^2 - sum_i ||u_i||^2) / 2,   S_c = sum_{y_i=c} u_i,
         with u_i = x_i/||x_i||, sum_i ||u_i||^2 = B.
  loss3 needs only cos(x_0, x_j): the first n_neg (<= C = 1000) negative pairs
        in lexicographic (i<j) order all come from row i=0 whenever row 0 has
        >= n_neg differing-label partners (host-verified, else exact host
        fallback).

Sharding: labels are snake-packed onto 8 cores (125 labels/core); xs rows are
permuted so a core owns all rows of its labels -> per-core sum_c ||S_c||^2
partials are scalars (no cross-core reduction / no collectives). y_bars is
row-sharded naturally. Host glue is only O(B) index math plus summing 8 small
output vectors.
"""

import os
import numpy as np

try:
    import concourse.bass as bass  # noqa: F401
except ImportError:
    import sys

    for p in ("/opt/trn_rl_repo", "/root/.axon_site/_ro/trn_rl_repo"):
        if os.path.isdir(p):
            sys.path.insert(0, p)
            break
    import concourse.bass as bass  # noqa: F401

import concourse.tile as tile
from concourse import bacc, mybir
from concourse.bass_utils import run_bass_kernel_spmd

B, D, C, NCORES = 4096, 512, 1000, 8
RCE = B // NCORES  # 512 CE rows per core
PAD = 640          # padded loss2 row capacity per core (5 x 128)
LBL = 128          # label slots per core (1000/8 = 125 used)

F32 = mybir.dt.float32
BF16 = mybir.dt.bfloat16
ALU = mybir.AluOpType
ACTF = mybir.ActivationFunctionType

LAST_EXEC_NS = None  # filled when BASS_TRACE=1 and profiling succeeds

_PROGRAM = None  # cached (nc) build


def _build_program():
    """One SPMD program, identical on all 8 cores; all per-core variation is
    carried by the input tensors."""
    nc = bacc.Bacc("TRN2", target_bir_lowering=False, debug=False,
                   num_devices=NCORES)

    NG = RCE // 128   # 4 row-groups for CE
    NX = PAD // 128   # 5 row-groups for loss2/loss3

    # packed per-core output columns: se[0:NG], pk[NG:2NG], cos0[2NG:2NG+NX],
    # snorm[2NG+NX]
    OCOLS = 2 * NG + NX + 1

    # big tensors staged in bf16: halves DMA bytes (memory-bound regime);
    # rel-err impact on the final scalar is ~1e-4, far inside tolerance
    yb = nc.declare_dram_parameter("yb", [RCE, C], BF16, isOutput=False)
    ycls = nc.declare_dram_parameter("ycls", [128, NG], F32, isOutput=False)
    xsp = nc.declare_dram_parameter("xsp", [PAD, D], BF16, isOutput=False)
    yrel = nc.declare_dram_parameter("yrel", [128, NX], F32, isOutput=False)
    u0b = nc.declare_dram_parameter("u0b", [128, D], BF16, isOutput=False)
    outp = nc.declare_dram_parameter("outp", [128, OCOLS], F32, isOutput=True)

    with tile.TileContext(nc) as tc:
        with (
            tc.tile_pool(name="const", bufs=1) as const,
            tc.tile_pool(name="io", bufs=1) as io,
            tc.tile_pool(name="trash", bufs=2) as trash,
            tc.tile_pool(name="small", bufs=4) as small,
            tc.tile_pool(name="psum", bufs=1, space="PSUM") as psum,
        ):
            iota_c = const.tile([128, C], F32)
            nc.gpsimd.iota(iota_c[:], [[1, C]], channel_multiplier=0,
                           allow_small_or_imprecise_dtypes=True)
            iota_l = const.tile([128, LBL], F32)
            nc.gpsimd.iota(iota_l[:], [[1, LBL]], channel_multiplier=0,
                           allow_small_or_imprecise_dtypes=True)
            op = small.tile([128, OCOLS], F32)

            # 3D-pattern chunked DMAs: row (n*128 + p) lands in partition p,
            # group n. Two chunks per big tensor so group-0 compute overlaps
            # the remaining transfer; small tensors are partition-major on
            # the host side (contiguous per-partition runs, no 4B scatter).
            xsp_r = xsp.rearrange("(n p) d -> p n d", p=128)
            yb_r = yb.rearrange("(n p) c -> p n c", p=128)
            u0t = const.tile([128, D], BF16)
            nc.sync.dma_start(u0t[:], u0b[:])
            yr = small.tile([128, NX], F32)
            nc.sync.dma_start(yr[:], yrel[:])
            yc = small.tile([128, NG], F32)
            nc.sync.dma_start(yc[:], ycls[:])
            # xsp fully first: the loss2 chain (norms -> sqrt -> one-hot ->
            # matmul) is the longest dependency chain, and putting Sqrt
            # before any Exp keeps ACT at 2 table loads.
            xt = io.tile([128, NX, D], BF16)
            nc.sync.dma_start(xt[:, 0:3, :], xsp_r[:, 0:3, :])
            nc.sync.dma_start(xt[:, 3:NX, :], xsp_r[:, 3:NX, :])
            t = io.tile([128, NG, C], BF16)
            nc.sync.dma_start(t[:, 0:2, :], yb_r[:, 0:2, :])
            nc.sync.dma_start(t[:, 2:NG, :], yb_r[:, 2:NG, :])

            # ---- loss2 norms: |x|^2 per row, one Sqrt/recip for all groups
            nqs = small.tile([128, NX], F32)
            for i in range(NX):
                td = trash.tile([128, D], BF16)
                nc.vector.scalar_tensor_tensor(
                    td[:], xt[:, i, :], 1.0, xt[:, i, :],
                    op0=ALU.mult, op1=ALU.mult, accum_out=nqs[:, i:i + 1])
            # floor |x|^2 at 1e-12: zero-padded rows get inv_n = 1e6 but
            # contribute x*inv_n = 0 everywhere downstream.
            nc.vector.tensor_scalar_max(nqs[:], nqs[:], 1e-12)
            nrm = small.tile([128, NX], F32)
            nc.scalar.activation(nrm[:], nqs[:], ACTF.Sqrt)
            invn = small.tile([128, NX], F32)
            nc.vector.reciprocal(invn[:], nrm[:])

            # ---- loss2 matmul + loss3 row-0 cosine -----------------------
            S = psum.tile([128, D], F32)
            dt0 = small.tile([128, NX], F32)
            for i in range(NX):
                # scaled one-hot: oh[p, l] = (l == yrel_p) * inv_n_p, so the
                # matmul S = oh^T @ x accumulates already-normalized rows.
                oh = small.tile([128, LBL], BF16)
                nc.vector.tensor_scalar(
                    oh[:], iota_l[:], yr[:, i:i + 1], invn[:, i:i + 1],
                    op0=ALU.is_equal, op1=ALU.mult)
                nc.tensor.matmul(S[:], oh[:], xt[:, i, :],
                                 start=(i == 0), stop=(i == NX - 1))

                # cos(x_0, x_j) = (x_j . u0) * inv_n_j for local rows
                td2 = trash.tile([128, D], BF16)
                nc.vector.scalar_tensor_tensor(
                    td2[:], xt[:, i, :], 1.0, u0t[:],
                    op0=ALU.mult, op1=ALU.mult, accum_out=dt0[:, i:i + 1])
                nc.vector.tensor_mul(op[:, 2 * NG + i:2 * NG + i + 1],
                                     dt0[:, i:i + 1], invn[:, i:i + 1])

            # ---- loss1: exp+rowsum (one ACT pass/group) + label gather ---
            for i in range(NG):
                ex = trash.tile([128, C], BF16)
                nc.scalar.activation(ex[:], t[:, i, :], ACTF.Exp,
                                     accum_out=op[:, i:i + 1])
                tt = trash.tile([128, C], BF16)
                nc.vector.scalar_tensor_tensor(
                    tt[:], iota_c[:], yc[:, i:i + 1], t[:, i, :],
                    op0=ALU.is_equal, op1=ALU.mult,
                    accum_out=op[:, NG + i:NG + i + 1])

            # ---- ||S_c||^2 (DVE may read only one PSUM operand) ----------
            Ssb = io.tile([128, D], F32)
            nc.vector.tensor_copy(Ssb[:], S[:])
            td3 = trash.tile([128, D], F32)
            nc.vector.scalar_tensor_tensor(
                td3[:], Ssb[:], 1.0, Ssb[:],
                op0=ALU.mult, op1=ALU.mult,
                accum_out=op[:, OCOLS - 1:OCOLS])

            nc.sync.dma_start(outp[:], op[:])

    nc.compile()
    return nc


def _get_program():
    global _PROGRAM
    if _PROGRAM is None:
        _PROGRAM = _build_program()
    return _PROGRAM


def _shard(xs, y_bars, y):
    """Snake-pack labels onto cores; permute xs rows into per-core blocks.

    Returns in_maps plus the bookkeeping needed to reassemble outputs, or
    None if capacity assumptions fail (handled by exact host fallback).
    """
    counts = np.bincount(y, minlength=C).astype(np.int64)
    order = np.argsort(-counts, kind="stable")  # labels by count desc
    core_of_label = np.empty(C, np.int32)
    local_idx = np.empty(C, np.int32)
    nlab = np.zeros(NCORES, np.int32)
    for r in range(0, C, NCORES):
        blk = order[r:r + NCORES]
        cores = range(NCORES) if (r // NCORES) % 2 == 0 else range(NCORES - 1, -1, -1)
        for lab, core in zip(blk, cores):
            core_of_label[lab] = core
            local_idx[lab] = nlab[core]
            nlab[core] += 1
    if nlab.max() > LBL:
        return None

    core_of_row = core_of_label[y]
    perm = np.argsort(core_of_row, kind="stable")
    rows_per_core = np.bincount(core_of_row, minlength=NCORES)
    if rows_per_core.max() > PAD:
        return None
    starts = np.concatenate([[0], np.cumsum(rows_per_core)])

    import ml_dtypes

    bf16 = ml_dtypes.bfloat16
    n0 = np.linalg.norm(xs[0].astype(np.float64))
    u0 = (xs[0] / max(n0, 1e-12)).astype(bf16)
    u0b = np.ascontiguousarray(np.broadcast_to(u0, (128, D)))

    in_maps = []
    for k in range(NCORES):
        rows = perm[starts[k]:starts[k + 1]]
        xsp = np.zeros((PAD, D), bf16)
        xsp[:len(rows)] = xs[rows].astype(bf16)
        yrel = np.zeros(PAD, np.float32)
        yrel[:len(rows)] = local_idx[y[rows]]
        sl = slice(k * RCE, (k + 1) * RCE)
        # ycls/yrel partition-major: [p, n] = value for row n*128 + p
        in_maps.append({
            "yb": np.ascontiguousarray(y_bars[sl].astype(bf16)),
            "ycls": np.ascontiguousarray(
                y[sl].astype(np.float32).reshape(-1, 128).T),
            "xsp": xsp,
            "yrel": np.ascontiguousarray(yrel.reshape(-1, 128).T),
            "u0b": u0b,
        })
    return in_maps, perm, rows_per_core, starts, counts


def _host_reference(xs, y_bars, y, n_negative):
    """Exact numpy replica of the jax reference (fallback only)."""
    logits = y_bars.astype(np.float64)
    m = logits.max(axis=1, keepdims=True)
    lse = np.log(np.exp(logits - m).sum(axis=1)) + m[:, 0]
    loss1 = np.mean(lse - logits[np.arange(B), y])

    norms = np.linalg.norm(xs.astype(np.float64), axis=-1)
    cos = (xs.astype(np.float64) @ xs.T.astype(np.float64)) / np.maximum(
        norms[:, None] * norms[None, :], 1e-8)
    same = y[:, None] == y[None, :]
    triu = np.triu(np.ones((B, B), bool), k=1)
    pos = same & triu
    n_pos = pos.sum()
    loss2 = ((1.0 - cos)[pos].sum() / max(n_pos, 1)) if n_pos > 0 else 0.0
    counts = np.bincount(y, minlength=C)
    n_neg = max(int(n_negative), int((counts > 1).sum()))
    neg_flat = (~same & triu).reshape(-1)
    rank = np.cumsum(neg_flat)
    sel = neg_flat & (rank <= n_neg)
    loss3 = np.maximum(cos.reshape(-1)[sel], 0.0).sum() / n_neg
    return np.float32(loss1 + loss2 + loss3)


def kernel(xs, y_bars, y_gths, n_negative):
    global LAST_EXEC_NS
    xs = np.ascontiguousarray(np.asarray(xs, np.float32))
    y_bars = np.ascontiguousarray(np.asarray(y_bars, np.float32))
    y = np.asarray(y_gths).astype(np.int64).reshape(-1)
    n_negative = int(np.asarray(n_negative).reshape(()))
    assert xs.shape == (B, D) and y_bars.shape == (B, C) and y.shape == (B,)

    sharded = _shard(xs, y_bars, y)
    counts = np.bincount(y, minlength=C)
    n_dup = int((counts > 1).sum())
    n_neg = max(n_negative, n_dup)
    # loss3 structural requirement: first n_neg lexicographic negative pairs
    # must all be (0, j) pairs.
    neg0 = int((y[1:] != y[0]).sum())
    if sharded is None or n_neg <= 0 or neg0 < n_neg:
        return _host_reference(xs, y_bars, y, n_negative)

    in_maps, perm, rows_per_core, starts, _ = sharded
    nc = _get_program()
    res = run_bass_kernel_spmd(nc, in_maps, core_ids=list(range(NCORES)),
                               trace=bool(os.environ.get("BASS_TRACE")))
    if res.exec_time_ns is not None:
        LAST_EXEC_NS = res.exec_time_ns

    NG, NX = RCE // 128, PAD // 128
    outs = [res.results[k]["outp"] for k in range(NCORES)]
    # packed columns: [p, n] holds row n*128 + p of the per-core vector
    se_all = np.concatenate([o[:, 0:NG].T.reshape(-1) for o in outs])
    pk_all = np.concatenate([o[:, NG:2 * NG].T.reshape(-1) for o in outs])
    loss1 = float((np.log(se_all.astype(np.float64)) - pk_all).mean())

    sum_sq = float(sum(o[:, -1].astype(np.float64).sum() for o in outs))
    n_pos = int((counts.astype(np.int64) * (counts.astype(np.int64) - 1) // 2).sum())
    pos_cos = (sum_sq - float(B)) / 2.0
    loss2 = (n_pos - pos_cos) / max(n_pos, 1) if n_pos > 0 else 0.0

    cos0_global = np.empty(B, np.float64)
    gathered = np.concatenate([
        outs[k][:, 2 * NG:2 * NG + NX].T.reshape(-1)[:rows_per_core[k]]
        for k in range(NCORES)])
    cos0_global[perm] = gathered
    neg_j = np.nonzero(y[1:] != y[0])[0][:n_neg] + 1
    loss3 = float(np.maximum(cos0_global[neg_j], 0.0).sum()) / n_neg

    return np.asarray(loss1 + loss2 + loss3, np.float32)
